# revision 1
# baseline (speedup 1.0000x reference)
"""CapsuleLayer (dynamic routing) Trainium2 kernel.

x: [128, 2048, 8] f32, W: [2048, 32, 8, 16] f32 -> v: [128, 32, 16] f32

Sharding: batch B=128 split across 8 cores (16 each), W replicated
(resident in SBUF, bf16). Per core, per routing pass, u_hat tiles
([128, 1024] = 16 caps x 16 batch x 512 (o,d)) are recomputed on the PE
via a block-diagonal-x matmul and consumed on-chip; u_hat never touches
HBM. Engine split: PE produce + softmax-weighted n-reduction (matmul
with block-ones lhsT), ACT PSUM drain + exp, DVE multiplies, Pool
d-reduce.
"""

from contextlib import ExitStack

import numpy as np
import ml_dtypes

import concourse.bass as bass
import concourse.bacc as bacc
import concourse.tile as tile
from concourse import mybir
from concourse.bass_utils import run_bass_kernel_spmd

BF16 = mybir.dt.bfloat16
F32 = mybir.dt.float32
X = mybir.AxisListType.X
Exp = mybir.ActivationFunctionType.Exp
Copy = mybir.ActivationFunctionType.Copy

B, N, O, I, D = 128, 2048, 32, 8, 16
CORES = 8
BL = B // CORES            # 16 batch elements per core
J2 = N // 16               # 128 blocks of 16 input caps
OD = O * D                 # 512
G = 4                      # j2 group size for batched softmax

_BF = ml_dtypes.bfloat16


def _bcast_last(ap, count):
    """Append a step-0 (broadcast) innermost dim to an AP."""
    return bass.AP(tensor=ap.tensor, offset=ap.offset, ap=list(ap.ap) + [[0, count]])


def build_nc():
    nc = bacc.Bacc("TRN2", target_bir_lowering=False)

    w = nc.dram_tensor("w", [J2, 128, OD], BF16, kind="ExternalInput")
    xt = nc.dram_tensor("xt", [J2, 128, BL], BF16, kind="ExternalInput")
    xbd = nc.dram_tensor("xbd", [J2, 128, 2 * 128], BF16, kind="ExternalInput")
    ones = nc.dram_tensor("ones", [128, 8], BF16, kind="ExternalInput")
    out = nc.dram_tensor("out", [BL, OD], F32, kind="ExternalOutput")

    with tile.TileContext(nc) as tc, ExitStack() as ctx:
        xbdp = ctx.enter_context(tc.tile_pool(name="xbdp", bufs=4))
        const = ctx.enter_context(tc.tile_pool(name="const", bufs=1))
        biasp = ctx.enter_context(tc.tile_pool(name="biasp", bufs=1))
        vexpp = ctx.enter_context(tc.tile_pool(name="vexpp", bufs=2))
        work = ctx.enter_context(tc.tile_pool(name="work", bufs=3))
        small = ctx.enter_context(tc.tile_pool(name="small", bufs=4))
        sqp = ctx.enter_context(tc.tile_pool(name="sqp", bufs=1))
        psum_u = ctx.enter_context(tc.tile_pool(name="psum_u", bufs=2, space="PSUM"))
        psum_s = ctx.enter_context(tc.tile_pool(name="psum_s", bufs=1, space="PSUM"))
        dramp = ctx.enter_context(tc.tile_pool(name="dramp", bufs=4, space="DRAM"))

        ones_sb = const.tile([128, 8], BF16)
        nc.sync.dma_start(out=ones_sb[:], in_=ones[:])
        xt_all = const.tile([128, J2, BL], BF16)
        nc.sync.dma_start(out=xt_all[:], in_=xt[:].rearrange("j p b -> p j b"))
        w_all = const.tile([128, J2, OD], BF16)
        w_r = w[:].rearrange("j p f -> p j f")
        for ch in range(8):
            nc.sync.dma_start(
                out=w_all[:, ch * 16 : (ch + 1) * 16, :],
                in_=w_r[:, ch * 16 : (ch + 1) * 16, :],
            )
        # bias[(n16 b8) partition, (j2, h, o)] f32
        bias_all = biasp.tile([128, J2, 2, O], F32)

        def squash(s_ap, P, v_ap):
            """v = s * |s|^2/(1+|s|^2) / sqrt(|s|^2 + 1e-8), per (b, o) over d."""
            s_sb = sqp.tile([P, OD], F32, tag="s_sb")
            nc.scalar.activation(s_sb[:], s_ap, Copy)
            ssq = sqp.tile([P, OD], F32, tag="ssq")
            nc.vector.tensor_mul(ssq[:], s_sb[:], s_sb[:])
            sq = sqp.tile([P, O], F32, tag="sq")
            nc.vector.reduce_sum(
                out=sq[:], in_=ssq[:].rearrange("p (o d) -> p o d", d=D), axis=X
            )
            d1 = sqp.tile([P, O], F32, tag="d1")
            nc.vector.tensor_scalar_add(d1[:], sq[:], 1.0)
            r1 = sqp.tile([P, O], F32, tag="r1")
            nc.vector.reciprocal(r1[:], d1[:])
            t = sqp.tile([P, O], F32, tag="t")
            nc.vector.tensor_mul(t[:], sq[:], r1[:])
            d2 = sqp.tile([P, O], F32, tag="d2")
            nc.vector.tensor_scalar_add(d2[:], sq[:], 1e-8)
            rt = sqp.tile([P, O], F32, tag="rt")
            nc.scalar.sqrt(rt[:], d2[:])
            rs = sqp.tile([P, O], F32, tag="rs")
            nc.vector.reciprocal(rs[:], rt[:])
            scale = sqp.tile([P, O], F32, tag="scale")
            nc.vector.tensor_mul(scale[:], t[:], rs[:])
            nc.vector.tensor_mul(
                v_ap.rearrange("p (o d) -> p o d", d=D),
                s_sb[:].rearrange("p (o d) -> p o d", d=D),
                _bcast_last(scale[:], D),
            )

        def make_vexp(parts):
            """parts: [(ap [rows, OD] bf16, dram row offset)] -> [128, 2*OD] tile.

            vexp[p=(n16 b8), h*OD + (o,d)] = v[h*8 + p%8, o, d]
            """
            vdram = dramp.tile([BL, OD], BF16, tag="vdram")
            for ap_, off in parts:
                nc.sync.dma_start(out=vdram[off : off + ap_.shape[0], :], in_=ap_)
            vx = vexpp.tile([128, 2 * OD], BF16, tag="vexp")
            for h in range(2):
                for g in range(16):
                    nc.sync.dma_start(
                        out=vx[g * 8 : (g + 1) * 8, h * OD : (h + 1) * OD],
                        in_=vdram[h * 8 : (h + 1) * 8, :],
                    )
            return vx

        # ---------------- pass 1 (iter 0): s0 = sum_n u_hat / 32 ----------------
        s0_ps = psum_s.tile([BL, OD], F32, tag="sacc0")
        for j2 in range(J2):
            nc.tensor.matmul(
                s0_ps[:],
                xt_all[:, j2, :],
                w_all[:, j2, :],
                start=(j2 == 0),
                stop=(j2 == J2 - 1),
            )
        v_bf = sqp.tile([BL, OD], BF16, tag="vbf")
        squash(s0_ps[:], BL, v_bf[:])
        vexp = make_vexp([(v_bf[:], 0)])

        # ---------------- passes 2, 3 (iters 1, 2) ----------------
        for k in (1, 2):
            # s accumulator [8, 2*OD]: cols h*OD+(o,d) for batch b = h*8 + row
            s_ps = psum_s.tile([8, 2 * OD], F32, name=f"sacc_{k}", tag="sacc1")
            for jg in range(J2 // G):
                usb_g = []
                for jj in range(G):
                    j2 = jg * G + jj
                    xbd_t = xbdp.tile([128, 2 * 128], BF16)
                    nc.sync.dma_start(out=xbd_t[:], in_=xbd[:][j2, :, :])
                    u_ps = psum_u.tile([128, 2 * OD], F32)
                    for h in range(2):
                        nc.tensor.matmul(
                            u_ps[:, h * OD : (h + 1) * OD],
                            xbd_t[:, h * 128 : (h + 1) * 128],
                            w_all[:, j2, :],
                            start=True,
                            stop=True,
                        )
                    u_sb = work.tile([128, 2 * OD], BF16, tag="usb", bufs=5)
                    nc.scalar.activation(u_sb[:], u_ps[:], Copy)
                    usb_g.append(u_sb)
                    # a[b,n,(h,o)] = sum_d u*v  (both halves in one op)
                    q = work.tile([128, 2 * OD], BF16, tag="q", bufs=3)
                    nc.vector.tensor_mul(q[:], u_sb[:], vexp[:])
                    bias_slice = bias_all[:, j2, :, :]  # [128, 2, O]
                    a_out = (
                        bias_slice
                        if k == 1
                        else small.tile([128, 2, O], F32, name="a2", tag="a2")[:]
                    )
                    nc.vector.reduce_sum(
                        out=a_out,
                        in_=q[:].rearrange("p (h o d) -> p h o d", h=2, d=D),
                        axis=X,
                    )
                    if k != 1:
                        nc.vector.tensor_add(bias_slice, bias_slice, a_out)
                # grouped softmax over the G j2's just processed
                bias_g = bias_all[:, jg * G : (jg + 1) * G, :, :]  # [128,G,2,O]
                ex = small.tile([128, G, 2, O], BF16, tag="ex")
                nc.scalar.activation(ex[:], bias_g, Exp)
                se = small.tile([128, G, 2], F32, tag="se")
                nc.vector.reduce_sum(out=se[:], in_=ex[:], axis=X)
                rse = small.tile([128, G, 2], F32, tag="rse")
                nc.vector.reciprocal(rse[:], se[:])
                c_t = small.tile([128, G, 2, O], BF16, tag="ct")
                nc.vector.tensor_mul(c_t[:], ex[:], _bcast_last(rse[:], O))
                for jj in range(G):
                    j2 = jg * G + jj
                    # e = u * c (c broadcast over d), s += ones^T @ e
                    e_t = work.tile([128, 2 * OD], BF16, tag="et", bufs=3)
                    nc.vector.tensor_mul(
                        e_t[:].rearrange("p (h o d) -> p h o d", h=2, d=D),
                        usb_g[jj][:].rearrange("p (h o d) -> p h o d", h=2, d=D),
                        _bcast_last(c_t[:, jj, :, :], D),
                    )
                    for h in range(2):
                        nc.tensor.matmul(
                            s_ps[:, h * OD : (h + 1) * OD],
                            ones_sb[:],
                            e_t[:, h * OD : (h + 1) * OD],
                            start=(j2 == 0),
                            stop=(j2 == J2 - 1),
                        )
            if k == 1:
                vtmps = []
                for h in range(2):
                    vtmp = sqp.tile([8, OD], BF16, tag="vtmp")
                    squash(s_ps[:, h * OD : (h + 1) * OD], 8, vtmp[:])
                    vtmps.append(vtmp)
                vexp = make_vexp([(vtmps[0][:], 0), (vtmps[1][:], 8)])
            else:
                for h in range(2):
                    v_f32 = sqp.tile([8, OD], F32, tag="vf32")
                    squash(s_ps[:, h * OD : (h + 1) * OD], 8, v_f32[:])
                    nc.sync.dma_start(
                        out=out[:][h * 8 : (h + 1) * 8, :], in_=v_f32[:]
                    )

    nc.compile()
    return nc


_nc_cache = {}


def _get_nc():
    if "nc" not in _nc_cache:
        _nc_cache["nc"] = build_nc()
    return _nc_cache["nc"]


def _prep_host(x, W):
    """Build the per-core input maps (numpy only)."""
    # W16[j2][(n,i)][(o,d)] = W[16*j2+n, o, i, d]
    W16 = (
        W.reshape(J2, 16, O, I, D)
        .transpose(0, 1, 3, 2, 4)
        .reshape(J2, 128, OD)
        .astype(_BF)
    )
    ones_bd = np.zeros((128, 8), dtype=_BF)
    for p in range(128):
        ones_bd[p, p % 8] = 1.0
    in_maps = []
    for c in range(CORES):
        xl = x[c * BL : (c + 1) * BL]  # [16, 2048, 8]
        T = xl.reshape(BL, J2, 16, I).transpose(1, 2, 3, 0)  # [j2, n, i, b]
        xt = (T / 32.0).reshape(J2, 128, BL).astype(_BF)
        xbd = np.zeros((J2, 128, 2, 128), dtype=np.float32)
        for n in range(16):
            xbd[:, n * 8 : (n + 1) * 8, 0, n * 8 : (n + 1) * 8] = T[:, n, :, 0:8]
            xbd[:, n * 8 : (n + 1) * 8, 1, n * 8 : (n + 1) * 8] = T[:, n, :, 8:16]
        in_maps.append(
            {
                "w": W16,
                "xt": xt,
                "xbd": xbd.reshape(J2, 128, 256).astype(_BF),
                "ones": ones_bd,
            }
        )
    return in_maps


TRACE = False
_last = {}


def kernel(x: np.ndarray, W: np.ndarray) -> np.ndarray:
    nc = _get_nc()
    in_maps = _prep_host(
        np.asarray(x, dtype=np.float32), np.asarray(W, dtype=np.float32)
    )
    res = run_bass_kernel_spmd(
        nc, in_maps, core_ids=list(range(CORES)), trace=TRACE
    )
    _last["res"] = res
    outs = [r["out"].reshape(BL, O, D) for r in res.results]
    return np.concatenate(outs, axis=0).astype(np.float32)


if __name__ == "__main__":
    rng = np.random.default_rng(0)
    x = rng.standard_normal((B, N, I), dtype=np.float32)
    W = rng.standard_normal((N, O, I, D), dtype=np.float32)
    v = kernel(x, W)
    print(v.shape, v.dtype, float(np.abs(v).mean()))



# revision 2
# speedup vs baseline: 1.0166x; 1.0166x over previous
"""CapsuleLayer (dynamic routing) Trainium2 kernel, v5 (pipelined emission).

x: [128, 2048, 8] f32, W: [2048, 32, 8, 16] f32 -> v: [128, 32, 16] f32

Batch split across 8 cores (16 each), W replicated in SBUF (bf16).
Layout: partition p = (n8, b16); u free = (nb2, d16, o32). Logits are
linear in v so pass 3 uses vexp(v1+v2); no bias tensor is stored.

Per j2 (16 input caps): PE produces u (two k=64 matmuls) and reduces n
(ones matmul); ACT drains PSUM and does exp; DVE does the two big
elementwise muls at 2x plus softmax smalls; the d-reduce splits between a
PE identity-matmul chain (nb=0, PSUM) and a gpsimd add-tree (nb=1, SBUF).

Instruction emission is software-pipelined in three stages (produce |
q+dred | softmax+e+ones) staggered by one j2-group each so no engine
sequencer blocks on a just-issued cross-engine dependency.
"""

from contextlib import ExitStack

import numpy as np
import ml_dtypes

import concourse.bass as bass
import concourse.bacc as bacc
import concourse.tile as tile
from concourse import mybir
from concourse.bass_utils import run_bass_kernel_spmd

BF16 = mybir.dt.bfloat16
F32 = mybir.dt.float32
X = mybir.AxisListType.X
Exp = mybir.ActivationFunctionType.Exp
Copy = mybir.ActivationFunctionType.Copy
MULT = mybir.AluOpType.mult
ADD = mybir.AluOpType.add

B, N, O, I, D = 128, 2048, 32, 8, 16
CORES = 8
BL = B // CORES            # 16 batch elements per core
J2 = N // 16               # 128 groups of 16 input caps
OD = O * D                 # 512
G = 4                      # j2 group size for batched softmax

_BF = ml_dtypes.bfloat16


def _ap(t_ap, extra_offset, dims):
    """AP with the same tensor/partition dim but custom free dims."""
    return bass.AP(
        tensor=t_ap.tensor,
        offset=t_ap.offset + extra_offset,
        ap=[list(t_ap.ap[0])] + [list(d) for d in dims],
    )


def build_nc():
    nc = bacc.Bacc("TRN2", target_bir_lowering=False)

    w8 = nc.dram_tensor("w8", [128, J2, OD], BF16, kind="ExternalInput")
    xt8 = nc.dram_tensor("xt8", [128, J2, BL], BF16, kind="ExternalInput")
    xbd8 = nc.dram_tensor("xbd8", [128, J2, 128], BF16, kind="ExternalInput")
    ones8 = nc.dram_tensor("ones8", [128, BL], BF16, kind="ExternalInput")
    ident = nc.dram_tensor("ident", [128, 128], BF16, kind="ExternalInput")
    out = nc.dram_tensor("out", [BL, OD], F32, kind="ExternalOutput")

    with tile.TileContext(nc) as tc, ExitStack() as ctx:
        const = ctx.enter_context(tc.tile_pool(name="const", bufs=1))
        xtp = ctx.enter_context(tc.tile_pool(name="xtp", bufs=2))
        xbp = ctx.enter_context(tc.tile_pool(name="xbp", bufs=4))
        vp = ctx.enter_context(tc.tile_pool(name="vp", bufs=1))
        usbp = ctx.enter_context(tc.tile_pool(name="usbp", bufs=11))
        qp = ctx.enter_context(tc.tile_pool(name="qp", bufs=2))
        tp = ctx.enter_context(tc.tile_pool(name="tp", bufs=3))
        ep = ctx.enter_context(tc.tile_pool(name="ep", bufs=3))
        smp = ctx.enter_context(tc.tile_pool(name="smp", bufs=3))
        sqp = ctx.enter_context(tc.tile_pool(name="sqp", bufs=1))
        psum_u = ctx.enter_context(tc.tile_pool(name="psum_u", bufs=2, space="PSUM"))
        psum_a = ctx.enter_context(tc.tile_pool(name="psum_a", bufs=2, space="PSUM"))
        psum_s = ctx.enter_context(tc.tile_pool(name="psum_s", bufs=1, space="PSUM"))

        w8_sb = const.tile([128, J2, OD], BF16)
        for ch in range(8):
            nc.sync.dma_start(
                out=w8_sb[:, ch * 16 : (ch + 1) * 16, :],
                in_=w8[:][:, ch * 16 : (ch + 1) * 16, :],
            )
        ones_sb = const.tile([128, BL], BF16)
        nc.sync.dma_start(out=ones_sb[:], in_=ones8[:])
        id_sb = const.tile([128, 128], BF16)
        nc.sync.dma_start(out=id_sb[:], in_=ident[:])

        # v holders ([16, (d, o)])
        v1_bf = vp.tile([BL, OD], BF16, tag="v1")
        v12_bf = vp.tile([BL, OD], BF16, tag="v12")

        def squash(s_f32, v_out):
            """v_out(bf16) = squash(s) over d; s_f32: [16, (d,o)] f32 sbuf."""
            ssq = sqp.tile([BL, D, O], F32, tag="ssq")
            nc.vector.tensor_mul(
                ssq[:],
                s_f32.rearrange("p (d o) -> p d o", o=O),
                s_f32.rearrange("p (d o) -> p d o", o=O),
            )
            sq = sqp.tile([BL, O], F32, tag="sq")
            nc.vector.reduce_sum(
                out=sq[:], in_=ssq[:].rearrange("p d o -> p o d"), axis=X
            )
            d1 = sqp.tile([BL, O], F32, tag="d1")
            nc.vector.tensor_scalar_add(d1[:], sq[:], 1.0)
            r1 = sqp.tile([BL, O], F32, tag="r1")
            nc.vector.reciprocal(r1[:], d1[:])
            t = sqp.tile([BL, O], F32, tag="t")
            nc.vector.tensor_mul(t[:], sq[:], r1[:])
            d2 = sqp.tile([BL, O], F32, tag="d2")
            nc.vector.tensor_scalar_add(d2[:], sq[:], 1e-8)
            rt = sqp.tile([BL, O], F32, tag="rt")
            nc.scalar.sqrt(rt[:], d2[:])
            rs = sqp.tile([BL, O], F32, tag="rs")
            nc.vector.reciprocal(rs[:], rt[:])
            scale = sqp.tile([BL, O], F32, tag="scale")
            nc.vector.tensor_mul(scale[:], t[:], rs[:])
            nc.vector.tensor_mul(
                v_out.rearrange("p (d o) -> p d o", o=O),
                s_f32.rearrange("p (d o) -> p d o", o=O),
                _ap(scale[:], 0, [[0, D], [1, O]]),
            )

        def make_vexp(v_bf):
            """vexp[p=(n8,b), (d,o)] = v[b, (d,o)]; 8 sbuf->sbuf DMAs."""
            vx = vp.tile([128, OD], BF16, tag="vexp")
            for k in range(8):
                nc.sync.dma_start(
                    out=vx[k * BL : (k + 1) * BL, :], in_=v_bf[:]
                )
            return vx

        # ---------------- pass 1: s0 = sum_n u_hat / 32 ----------------
        s0_full = psum_s.tile([BL, 2, OD], F32, tag="sacc2")
        s0_ps = s0_full[:, 0, :]
        for ch in range(8):
            xt_t = xtp.tile([128, 16, BL], BF16)
            nc.sync.dma_start(
                out=xt_t[:], in_=xt8[:][:, ch * 16 : (ch + 1) * 16, :]
            )
            for j in range(16):
                j2 = ch * 16 + j
                nc.tensor.matmul(
                    s0_ps,
                    xt_t[:, j, :],
                    w8_sb[:, j2, :],
                    start=(j2 == 0),
                    stop=(j2 == J2 - 1),
                )
        s0_sb = sqp.tile([BL, OD], F32, tag="scopy")
        nc.scalar.activation(s0_sb[:], s0_ps, Copy)
        squash(s0_sb[:], v1_bf[:])
        vexp = make_vexp(v1_bf)

        # ---------------- passes 2, 3 (3-stage pipelined emission) ------
        NG = J2 // G
        for k in (1, 2):
            s_ps = psum_s.tile([BL, 2, OD], F32, name=f"sacc_{k}", tag="sacc2")
            state = {}

            def stage_a(g):
                """xbd loads + produce u + drain, for group g."""
                st = {"u": [], "xb": []}
                xb = xbp.tile([128, G, 128], BF16, tag="xb", name="xb")
                nc.sync.dma_start(
                    out=xb[:], in_=xbd8[:][:, g * G : (g + 1) * G, :]
                )
                for jj in range(G):
                    j2 = g * G + jj
                    u_ps = psum_u.tile([128, 2, D, O], F32)
                    for nb in range(2):
                        sl = slice(nb * 64, (nb + 1) * 64)
                        nc.tensor.matmul(
                            u_ps[:, nb, :, :],
                            xb[sl, jj, :],
                            w8_sb[sl, j2, :],
                            start=True,
                            stop=True,
                        )
                    u_sb = usbp.tile([128, 2, D, O], BF16, tag="usb")
                    nc.scalar.activation(u_sb[:], u_ps[:], Copy)
                    st["u"].append(u_sb)
                state[("a", g)] = st

            def stage_b(g):
                """q-mul, d-reduce (PE nb0 / Pool nb1), exp, for group g."""
                st = state[("a", g)]
                ex = smp.tile([128, G, 2, O], BF16, tag="ex")
                a1g = smp.tile([128, G, O], F32, tag="a1g")
                q = qp.tile([128, G, 2, D, O], BF16, tag="q")
                for jj in range(G):
                    nc.vector.tensor_mul(
                        q[:, jj],
                        st["u"][jj][:],
                        _ap(vexp[:], 0, [[0, 2], [O, D], [1, O]]),
                    )
                # nb=1 d-reduce: group-batched DVE add-tree (2x mode)
                t1 = tp.tile([128, G, 8, O], BF16, tag="t1")
                nc.vector.tensor_add(t1[:], q[:, :, 1, 0:8, :], q[:, :, 1, 8:16, :])
                t2 = tp.tile([128, G, 4, O], BF16, tag="t2")
                nc.vector.tensor_add(t2[:], t1[:, :, 0:4, :], t1[:, :, 4:8, :])
                t3 = tp.tile([128, G, 2, O], BF16, tag="t3")
                nc.vector.tensor_add(t3[:], t2[:, :, 0:2, :], t2[:, :, 2:4, :])
                nc.vector.tensor_add(a1g[:], t3[:, :, 0, :], t3[:, :, 1, :])
                # nb=0 d-reduce: PE identity-matmul chains + per-j2 exp
                for jj in range(G):
                    a_ps = psum_a.tile([128, 512], F32, tag="aps")
                    for dd in range(D):
                        nc.tensor.matmul(
                            a_ps[:, 0:O],
                            id_sb[:],
                            q[:, jj, 0, dd, :],
                            start=(dd == 0),
                            stop=(dd == D - 1),
                        )
                    nc.scalar.activation(ex[:, jj, 0, :], a_ps[:, 0:O], Exp)
                st["ex"], st["a1g"] = ex, a1g

            def stage_c(g):
                """softmax normalize + e-mul + n-reduce, for group g."""
                st = state.pop(("a", g))
                ex, a1g = st["ex"], st["a1g"]
                nc.scalar.activation(ex[:, :, 1, :], a1g[:], Exp)
                se = smp.tile([128, G, 2], F32, tag="se")
                nc.vector.reduce_sum(out=se[:], in_=ex[:], axis=X)
                rse = smp.tile([128, G, 2], F32, tag="rse")
                nc.vector.reciprocal(rse[:], se[:])
                c_t = smp.tile([128, G, 2, O], BF16, tag="ct")
                nc.vector.tensor_mul(
                    c_t[:], ex[:], _ap(rse[:], 0, [[2, G], [1, 2], [0, O]])
                )
                for jj in range(G):
                    j2 = g * G + jj
                    e_t = ep.tile([128, 2, D, O], BF16, tag="et")
                    nc.vector.tensor_mul(
                        e_t[:],
                        st["u"][jj][:],
                        _ap(c_t[:], jj * 2 * O, [[O, 2], [0, D], [1, O]]),
                    )
                    for nb in range(2):
                        nc.tensor.matmul(
                            s_ps[:, nb, :],
                            ones_sb[:],
                            e_t[:, nb, :, :],
                            start=(j2 == 0),
                            stop=(j2 == J2 - 1),
                        )

            for g in range(NG + 2):
                if g >= 2:
                    stage_c(g - 2)
                if 1 <= g <= NG:
                    stage_b(g - 1)
                if g < NG:
                    stage_a(g)

            # s = nb0 + nb1, squash (only one PSUM operand allowed per op)
            s1_sb = sqp.tile([BL, OD], F32, tag="s1copy")
            nc.scalar.activation(s1_sb[:], s_ps[:, 1, :], Copy)
            s_sb = sqp.tile([BL, OD], F32, tag="scopy")
            nc.vector.tensor_add(s_sb[:], s_ps[:, 0, :], s1_sb[:])
            if k == 1:
                v2_bf = vp.tile([BL, OD], BF16, tag="v2")
                squash(s_sb[:], v2_bf[:])
                nc.vector.tensor_add(v12_bf[:], v1_bf[:], v2_bf[:])
                vexp = make_vexp(v12_bf)
            else:
                v_f32 = sqp.tile([BL, OD], F32, tag="vf32")
                squash(s_sb[:], v_f32[:])
                nc.sync.dma_start(out=out[:], in_=v_f32[:])

    nc.compile()
    return nc


_nc_cache = {}


def _get_nc():
    if "nc" not in _nc_cache:
        _nc_cache["nc"] = build_nc()
    return _nc_cache["nc"]


def _prep_host(x, W):
    """Build the per-core input maps (numpy only)."""
    arrW = W.reshape(J2, 2, 8, O, I, D)  # [j2, nb, n8, o, i, d]
    W8h = arrW.transpose(1, 2, 4, 0, 5, 3).reshape(128, J2, OD).astype(_BF)
    ones8h = np.zeros((128, BL), dtype=_BF)
    for p in range(128):
        ones8h[p, p % BL] = 1.0
    identh = np.eye(128, dtype=_BF)
    in_maps = []
    for c in range(CORES):
        xl = x[c * BL : (c + 1) * BL]  # [16, 2048, 8]
        arr = xl.reshape(BL, J2, 2, 8, I)  # [b, j2, nb, n8, i]
        t = arr.transpose(2, 3, 4, 1, 0)  # [nb, n8, i, j2, b]
        xt8h = (t / 32.0).reshape(128, J2, BL).astype(_BF)
        xbd = np.zeros((2, 8, I, J2, 8, BL), dtype=np.float32)
        for n8 in range(8):
            xbd[:, n8, :, :, n8, :] = t[:, n8]
        xbd8h = xbd.reshape(128, J2, 128).astype(_BF)
        in_maps.append(
            {
                "w8": W8h,
                "xt8": xt8h,
                "xbd8": xbd8h,
                "ones8": ones8h,
                "ident": identh,
            }
        )
    return in_maps


TRACE = False
_last = {}


def kernel(x: np.ndarray, W: np.ndarray) -> np.ndarray:
    nc = _get_nc()
    in_maps = _prep_host(
        np.asarray(x, dtype=np.float32), np.asarray(W, dtype=np.float32)
    )
    res = run_bass_kernel_spmd(
        nc, in_maps, core_ids=list(range(CORES)), trace=TRACE
    )
    _last["res"] = res
    outs = [
        r["out"].reshape(BL, D, O).transpose(0, 2, 1) for r in res.results
    ]
    return np.concatenate(outs, axis=0).astype(np.float32)


if __name__ == "__main__":
    rng = np.random.default_rng(0)
    x = rng.standard_normal((B, N, I), dtype=np.float32)
    W = rng.standard_normal((N, O, I, D), dtype=np.float32)
    v = kernel(x, W)
    print(v.shape, v.dtype, float(np.abs(v).mean()))


# revision 3
# speedup vs baseline: 1.1752x; 1.1560x over previous
"""CapsuleLayer (dynamic routing) Trainium2 kernel, v7 (pipelined emission).

x: [128, 2048, 8] f32, W: [2048, 32, 8, 16] f32 -> v: [128, 32, 16] f32

Batch split across 8 cores (16 each), W replicated in SBUF (bf16).
Layout: partition p = (n8, b16); u free = (nb2, d16, o32). Logits are
linear in v so pass 3 uses vexp(v1+v2); no bias tensor is stored.

Per j2 (16 input caps): PE produces u (two k=64 matmuls) and reduces n
(ones matmul); ACT drains PSUM and does exp; DVE does the two big
elementwise muls at 2x plus softmax smalls; the d-reduce splits between a
PE identity-matmul chain (nb=0, PSUM) and a gpsimd add-tree (nb=1, SBUF).

Instruction emission is software-pipelined in three stages (produce |
q+dred | softmax+e+ones) staggered by one j2-group each so no engine
sequencer blocks on a just-issued cross-engine dependency.
"""

from contextlib import ExitStack

import numpy as np
import ml_dtypes

import concourse.bass as bass
import concourse.bacc as bacc
import concourse.tile as tile
from concourse import mybir
from concourse.bass_utils import run_bass_kernel_spmd

BF16 = mybir.dt.bfloat16
F32 = mybir.dt.float32
X = mybir.AxisListType.X
Exp = mybir.ActivationFunctionType.Exp
Copy = mybir.ActivationFunctionType.Copy
MULT = mybir.AluOpType.mult
ADD = mybir.AluOpType.add

B, N, O, I, D = 128, 2048, 32, 8, 16
CORES = 8
BL = B // CORES            # 16 batch elements per core
J2 = N // 16               # 128 groups of 16 input caps
OD = O * D                 # 512
G = 4                      # j2 group size for batched softmax

_BF = ml_dtypes.bfloat16


def _ap(t_ap, extra_offset, dims):
    """AP with the same tensor/partition dim but custom free dims."""
    return bass.AP(
        tensor=t_ap.tensor,
        offset=t_ap.offset + extra_offset,
        ap=[list(t_ap.ap[0])] + [list(d) for d in dims],
    )


def build_nc():
    nc = bacc.Bacc("TRN2", target_bir_lowering=False)

    w8 = nc.dram_tensor("w8", [128, J2, OD], BF16, kind="ExternalInput")
    xt8 = nc.dram_tensor("xt8", [128, J2, BL], BF16, kind="ExternalInput")
    xbd8 = nc.dram_tensor("xbd8", [128, J2, 128], BF16, kind="ExternalInput")
    ones8 = nc.dram_tensor("ones8", [128, BL], BF16, kind="ExternalInput")
    ident = nc.dram_tensor("ident", [128, 128], BF16, kind="ExternalInput")
    out = nc.dram_tensor("out", [BL, OD], F32, kind="ExternalOutput")

    with tile.TileContext(nc) as tc, ExitStack() as ctx:
        const = ctx.enter_context(tc.tile_pool(name="const", bufs=1))
        xtp = ctx.enter_context(tc.tile_pool(name="xtp", bufs=2))
        xbp = ctx.enter_context(tc.tile_pool(name="xbp", bufs=4))
        vp = ctx.enter_context(tc.tile_pool(name="vp", bufs=1))
        usbp = ctx.enter_context(tc.tile_pool(name="usbp", bufs=11))
        qp = ctx.enter_context(tc.tile_pool(name="qp", bufs=2))
        tp = ctx.enter_context(tc.tile_pool(name="tp", bufs=3))
        ep = ctx.enter_context(tc.tile_pool(name="ep", bufs=3))
        smp = ctx.enter_context(tc.tile_pool(name="smp", bufs=3))
        sqp = ctx.enter_context(tc.tile_pool(name="sqp", bufs=1))
        psum_u = ctx.enter_context(tc.tile_pool(name="psum_u", bufs=2, space="PSUM"))
        psum_a = ctx.enter_context(tc.tile_pool(name="psum_a", bufs=2, space="PSUM"))
        psum_s = ctx.enter_context(tc.tile_pool(name="psum_s", bufs=1, space="PSUM"))

        w8_chunks = []
        for ch in range(8):
            w8_c = const.tile([128, 16, OD], BF16, name=f"w8_{ch}", tag=f"w8_{ch}")
            w8_chunks.append(w8_c)

        def w8_sl(j2, sl=slice(None)):
            return w8_chunks[j2 // 16][sl, j2 % 16, :]
        ones_sb = const.tile([128, BL], BF16)
        nc.sync.dma_start(out=ones_sb[:], in_=ones8[:])
        id_sb = const.tile([128, 128], BF16)
        nc.sync.dma_start(out=id_sb[:], in_=ident[:])

        # v holders ([16, (d, o)])
        v1_bf = vp.tile([BL, OD], BF16, tag="v1")
        v12_bf = vp.tile([BL, OD], BF16, tag="v12")

        def squash(s_f32, v_out):
            """v_out(bf16) = squash(s) over d; s_f32: [16, (d,o)] f32 sbuf."""
            ssq = sqp.tile([BL, D, O], F32, tag="ssq")
            nc.vector.tensor_mul(
                ssq[:],
                s_f32.rearrange("p (d o) -> p d o", o=O),
                s_f32.rearrange("p (d o) -> p d o", o=O),
            )
            sq = sqp.tile([BL, O], F32, tag="sq")
            nc.vector.reduce_sum(
                out=sq[:], in_=ssq[:].rearrange("p d o -> p o d"), axis=X
            )
            d1 = sqp.tile([BL, O], F32, tag="d1")
            nc.vector.tensor_scalar_add(d1[:], sq[:], 1.0)
            r1 = sqp.tile([BL, O], F32, tag="r1")
            nc.vector.reciprocal(r1[:], d1[:])
            t = sqp.tile([BL, O], F32, tag="t")
            nc.vector.tensor_mul(t[:], sq[:], r1[:])
            d2 = sqp.tile([BL, O], F32, tag="d2")
            nc.vector.tensor_scalar_add(d2[:], sq[:], 1e-8)
            rt = sqp.tile([BL, O], F32, tag="rt")
            nc.scalar.sqrt(rt[:], d2[:])
            rs = sqp.tile([BL, O], F32, tag="rs")
            nc.vector.reciprocal(rs[:], rt[:])
            scale = sqp.tile([BL, O], F32, tag="scale")
            nc.vector.tensor_mul(scale[:], t[:], rs[:])
            nc.vector.tensor_mul(
                v_out.rearrange("p (d o) -> p d o", o=O),
                s_f32.rearrange("p (d o) -> p d o", o=O),
                _ap(scale[:], 0, [[0, D], [1, O]]),
            )

        def make_vexp(v_bf):
            """vexp[p=(n8,b), (d,o)] = v[b, (d,o)]; 8 sbuf->sbuf DMAs."""
            vx = vp.tile([128, OD], BF16, tag="vexp")
            for k in range(8):
                nc.sync.dma_start(
                    out=vx[k * BL : (k + 1) * BL, :], in_=v_bf[:]
                )
            return vx

        # ---------------- pass 1: s0 = sum_n u_hat / 32 ----------------
        s0_full = psum_s.tile([BL, 2, OD], F32, tag="sacc2")
        s0_ps = s0_full[:, 0, :]
        for ch in range(8):
            xt_t = xtp.tile([128, 16, BL], BF16)
            nc.sync.dma_start(
                out=xt_t[:], in_=xt8[:][:, ch * 16 : (ch + 1) * 16, :]
            )
            nc.sync.dma_start(
                out=w8_chunks[ch][:],
                in_=w8[:][:, ch * 16 : (ch + 1) * 16, :],
            )
            for j in range(16):
                j2 = ch * 16 + j
                nc.tensor.matmul(
                    s0_ps,
                    xt_t[:, j, :],
                    w8_sl(j2),
                    start=(j2 == 0),
                    stop=(j2 == J2 - 1),
                )
        s0_sb = sqp.tile([BL, OD], F32, tag="scopy")
        nc.scalar.activation(s0_sb[:], s0_ps, Copy)
        squash(s0_sb[:], v1_bf[:])
        vexp = make_vexp(v1_bf)

        # ---------------- passes 2, 3 (3-stage pipelined emission) ------
        NG = J2 // G
        for k in (1, 2):
            s_ps = psum_s.tile([BL, 2, OD], F32, name=f"sacc_{k}", tag="sacc2")
            state = {}

            def stage_a(g):
                """xbd loads + produce u + drain, for group g."""
                st = {"u": [], "xb": []}
                xb = xbp.tile([128, G, 128], BF16, tag="xb", name="xb")
                nc.sync.dma_start(
                    out=xb[:], in_=xbd8[:][:, g * G : (g + 1) * G, :]
                )
                for jj in range(G):
                    j2 = g * G + jj
                    u_ps = psum_u.tile([128, 2, D, O], F32)
                    for nb in range(2):
                        sl = slice(nb * 64, (nb + 1) * 64)
                        nc.tensor.matmul(
                            u_ps[:, nb, :, :],
                            xb[sl, jj, :],
                            w8_sl(j2, sl),
                            start=True,
                            stop=True,
                        )
                    u_sb = usbp.tile([128, 2, D, O], BF16, tag="usb")
                    nc.scalar.activation(u_sb[:], u_ps[:], Copy)
                    st["u"].append(u_sb)
                state[("a", g)] = st

            def stage_b(g):
                """q-mul, d-reduce (PE nb0 / Pool nb1), exp, for group g."""
                st = state[("a", g)]
                ex = smp.tile([128, G, 2, O], BF16, tag="ex")
                a1g = smp.tile([128, G, O], BF16, tag="a1g")
                q = qp.tile([128, G, 2, D, O], BF16, tag="q")
                for jj in range(G):
                    nc.vector.tensor_mul(
                        q[:, jj],
                        st["u"][jj][:],
                        _ap(vexp[:], 0, [[0, 2], [O, D], [1, O]]),
                    )
                # nb=1 d-reduce: group-batched DVE add-tree (2x mode)
                t1 = tp.tile([128, G, 8, O], BF16, tag="t1")
                nc.vector.tensor_add(t1[:], q[:, :, 1, 0:8, :], q[:, :, 1, 8:16, :])
                t2 = tp.tile([128, G, 4, O], BF16, tag="t2")
                nc.vector.tensor_add(t2[:], t1[:, :, 0:4, :], t1[:, :, 4:8, :])
                t3 = tp.tile([128, G, 2, O], BF16, tag="t3")
                nc.vector.tensor_add(t3[:], t2[:, :, 0:2, :], t2[:, :, 2:4, :])
                nc.vector.tensor_add(a1g[:], t3[:, :, 0, :], t3[:, :, 1, :])
                # nb=0 d-reduce: PE identity-matmul chains + per-j2 exp
                for jj in range(G):
                    a_ps = psum_a.tile([128, 512], F32, tag="aps")
                    for dd in range(D):
                        nc.tensor.matmul(
                            a_ps[:, 0:O],
                            id_sb[:],
                            q[:, jj, 0, dd, :],
                            start=(dd == 0),
                            stop=(dd == D - 1),
                        )
                    nc.scalar.activation(ex[:, jj, 0, :], a_ps[:, 0:O], Exp)
                st["ex"], st["a1g"] = ex, a1g

            def stage_c(g):
                """softmax normalize + e-mul + n-reduce, for group g."""
                st = state.pop(("a", g))
                ex, a1g = st["ex"], st["a1g"]
                nc.scalar.activation(ex[:, :, 1, :], a1g[:], Exp)
                se = smp.tile([128, G, 2], F32, tag="se")
                nc.vector.reduce_sum(out=se[:], in_=ex[:], axis=X)
                rse = smp.tile([128, G, 2], F32, tag="rse")
                nc.vector.reciprocal(rse[:], se[:])
                c_t = smp.tile([128, G, 2, O], BF16, tag="ct")
                nc.gpsimd.tensor_mul(
                    c_t[:], ex[:], _ap(rse[:], 0, [[2, G], [1, 2], [0, O]])
                )
                for jj in range(G):
                    j2 = g * G + jj
                    e_t = ep.tile([128, 2, D, O], BF16, tag="et")
                    nc.vector.tensor_mul(
                        e_t[:],
                        st["u"][jj][:],
                        _ap(c_t[:], jj * 2 * O, [[O, 2], [0, D], [1, O]]),
                    )
                    for nb in range(2):
                        nc.tensor.matmul(
                            s_ps[:, nb, :],
                            ones_sb[:],
                            e_t[:, nb, :, :],
                            start=(j2 == 0),
                            stop=(j2 == J2 - 1),
                        )

            for g in range(NG + 2):
                if g >= 2:
                    stage_c(g - 2)
                if 1 <= g <= NG:
                    stage_b(g - 1)
                if g < NG:
                    stage_a(g)

            # s = nb0 + nb1, squash (only one PSUM operand allowed per op)
            s1_sb = sqp.tile([BL, OD], F32, tag="s1copy")
            nc.scalar.activation(s1_sb[:], s_ps[:, 1, :], Copy)
            s_sb = sqp.tile([BL, OD], F32, tag="scopy")
            nc.vector.tensor_add(s_sb[:], s_ps[:, 0, :], s1_sb[:])
            if k == 1:
                v2_bf = vp.tile([BL, OD], BF16, tag="v2")
                squash(s_sb[:], v2_bf[:])
                nc.vector.tensor_add(v12_bf[:], v1_bf[:], v2_bf[:])
                vexp = make_vexp(v12_bf)
            else:
                v_f32 = sqp.tile([BL, OD], F32, tag="vf32")
                squash(s_sb[:], v_f32[:])
                nc.sync.dma_start(out=out[:], in_=v_f32[:])

    nc.compile()
    return nc


_nc_cache = {}


def _get_nc():
    if "nc" not in _nc_cache:
        _nc_cache["nc"] = build_nc()
    return _nc_cache["nc"]


def _prep_host(x, W):
    """Build the per-core input maps (numpy only)."""
    arrW = W.reshape(J2, 2, 8, O, I, D)  # [j2, nb, n8, o, i, d]
    W8h = arrW.transpose(1, 2, 4, 0, 5, 3).reshape(128, J2, OD).astype(_BF)
    ones8h = np.zeros((128, BL), dtype=_BF)
    for p in range(128):
        ones8h[p, p % BL] = 1.0
    identh = np.eye(128, dtype=_BF)
    in_maps = []
    for c in range(CORES):
        xl = x[c * BL : (c + 1) * BL]  # [16, 2048, 8]
        arr = xl.reshape(BL, J2, 2, 8, I)  # [b, j2, nb, n8, i]
        t = arr.transpose(2, 3, 4, 1, 0)  # [nb, n8, i, j2, b]
        xt8h = (t / 32.0).reshape(128, J2, BL).astype(_BF)
        xbd = np.zeros((2, 8, I, J2, 8, BL), dtype=np.float32)
        for n8 in range(8):
            xbd[:, n8, :, :, n8, :] = t[:, n8]
        xbd8h = xbd.reshape(128, J2, 128).astype(_BF)
        in_maps.append(
            {
                "w8": W8h,
                "xt8": xt8h,
                "xbd8": xbd8h,
                "ones8": ones8h,
                "ident": identh,
            }
        )
    return in_maps


TRACE = False
_last = {}


def kernel(x: np.ndarray, W: np.ndarray) -> np.ndarray:
    nc = _get_nc()
    in_maps = _prep_host(
        np.asarray(x, dtype=np.float32), np.asarray(W, dtype=np.float32)
    )
    res = run_bass_kernel_spmd(
        nc, in_maps, core_ids=list(range(CORES)), trace=TRACE
    )
    _last["res"] = res
    outs = [
        r["out"].reshape(BL, D, O).transpose(0, 2, 1) for r in res.results
    ]
    return np.concatenate(outs, axis=0).astype(np.float32)


if __name__ == "__main__":
    rng = np.random.default_rng(0)
    x = rng.standard_normal((B, N, I), dtype=np.float32)
    W = rng.standard_normal((N, O, I, D), dtype=np.float32)
    v = kernel(x, W)
    print(v.shape, v.dtype, float(np.abs(v).mean()))


# revision 4
# speedup vs baseline: 1.2123x; 1.0315x over previous
"""CapsuleLayer (dynamic routing) Trainium2 kernel, v10 (pipelined emission).

x: [128, 2048, 8] f32, W: [2048, 32, 8, 16] f32 -> v: [128, 32, 16] f32

Batch split across 8 cores (16 each), W replicated in SBUF (bf16).
Layout: partition p = (n8, b16); u free = (nb2, d16, o32). Logits are
linear in v so pass 3 uses vexp(v1+v2); no bias tensor is stored.

Per j2 (16 input caps): PE produces u (two k=64 matmuls) and reduces n
(ones matmul); ACT drains PSUM and does exp; DVE does the two big
elementwise muls at 2x plus softmax smalls; the d-reduce splits between a
PE identity-matmul chain (nb=0, PSUM) and a gpsimd add-tree (nb=1, SBUF).

Instruction emission is software-pipelined in three stages (produce |
q+dred | softmax+e+ones) staggered by one j2-group each so no engine
sequencer blocks on a just-issued cross-engine dependency.
"""

from contextlib import ExitStack

import numpy as np
import ml_dtypes

import concourse.bass as bass
import concourse.bacc as bacc
import concourse.tile as tile
from concourse import mybir
from concourse.bass_utils import run_bass_kernel_spmd

BF16 = mybir.dt.bfloat16
F32 = mybir.dt.float32
X = mybir.AxisListType.X
Exp = mybir.ActivationFunctionType.Exp
Copy = mybir.ActivationFunctionType.Copy
MULT = mybir.AluOpType.mult
ADD = mybir.AluOpType.add

B, N, O, I, D = 128, 2048, 32, 8, 16
CORES = 8
BL = B // CORES            # 16 batch elements per core
J2 = N // 16               # 128 groups of 16 input caps
OD = O * D                 # 512
G = 4                      # j2 group size for batched softmax

_BF = ml_dtypes.bfloat16


def _ap(t_ap, extra_offset, dims):
    """AP with the same tensor/partition dim but custom free dims."""
    return bass.AP(
        tensor=t_ap.tensor,
        offset=t_ap.offset + extra_offset,
        ap=[list(t_ap.ap[0])] + [list(d) for d in dims],
    )


def build_nc():
    nc = bacc.Bacc("TRN2", target_bir_lowering=False)

    w8 = nc.dram_tensor("w8", [128, J2, OD], BF16, kind="ExternalInput")
    xt8 = nc.dram_tensor("xt8", [128, J2, BL], BF16, kind="ExternalInput")
    xbd8 = nc.dram_tensor("xbd8", [128, J2, 128], BF16, kind="ExternalInput")
    ones8 = nc.dram_tensor("ones8", [128, BL], BF16, kind="ExternalInput")
    ident = nc.dram_tensor("ident", [128, 128], BF16, kind="ExternalInput")
    out = nc.dram_tensor("out", [BL, OD], F32, kind="ExternalOutput")

    with tile.TileContext(nc) as tc, ExitStack() as ctx:
        const = ctx.enter_context(tc.tile_pool(name="const", bufs=1))
        xtp = ctx.enter_context(tc.tile_pool(name="xtp", bufs=2))
        xbp = ctx.enter_context(tc.tile_pool(name="xbp", bufs=4))
        vp = ctx.enter_context(tc.tile_pool(name="vp", bufs=1))
        usbp = ctx.enter_context(tc.tile_pool(name="usbp", bufs=11))
        qp = ctx.enter_context(tc.tile_pool(name="qp", bufs=2))
        tp = ctx.enter_context(tc.tile_pool(name="tp", bufs=3))
        ep = ctx.enter_context(tc.tile_pool(name="ep", bufs=3))
        smp = ctx.enter_context(tc.tile_pool(name="smp", bufs=3))
        sqp = ctx.enter_context(tc.tile_pool(name="sqp", bufs=1))
        psum_u = ctx.enter_context(tc.tile_pool(name="psum_u", bufs=2, space="PSUM"))
        psum_a = ctx.enter_context(tc.tile_pool(name="psum_a", bufs=2, space="PSUM"))
        psum_s = ctx.enter_context(tc.tile_pool(name="psum_s", bufs=1, space="PSUM"))

        w8_chunks = []
        for ch in range(8):
            w8_c = const.tile([128, 16, OD], BF16, name=f"w8_{ch}", tag=f"w8_{ch}")
            w8_chunks.append(w8_c)

        def w8_sl(j2, sl=slice(None)):
            return w8_chunks[j2 // 16][sl, j2 % 16, :]
        ones_sb = const.tile([128, BL], BF16)
        nc.sync.dma_start(out=ones_sb[:], in_=ones8[:])
        id_sb = const.tile([128, 128], BF16)
        nc.sync.dma_start(out=id_sb[:], in_=ident[:])

        # v holders ([16, (d, o)])
        v1_bf = vp.tile([BL, OD], BF16, tag="v1")
        v12_bf = vp.tile([BL, OD], BF16, tag="v12")

        def squash(s_f32, v_out):
            """v_out(bf16) = squash(s) over d; s_f32: [16, (d,o)] f32 sbuf."""
            ssq = sqp.tile([BL, D, O], F32, tag="ssq")
            nc.vector.tensor_mul(
                ssq[:],
                s_f32.rearrange("p (d o) -> p d o", o=O),
                s_f32.rearrange("p (d o) -> p d o", o=O),
            )
            sq = sqp.tile([BL, O], F32, tag="sq")
            nc.vector.reduce_sum(
                out=sq[:], in_=ssq[:].rearrange("p d o -> p o d"), axis=X
            )
            d1 = sqp.tile([BL, O], F32, tag="d1")
            nc.vector.tensor_scalar_add(d1[:], sq[:], 1.0)
            r1 = sqp.tile([BL, O], F32, tag="r1")
            nc.vector.reciprocal(r1[:], d1[:])
            t = sqp.tile([BL, O], F32, tag="t")
            nc.vector.tensor_mul(t[:], sq[:], r1[:])
            d2 = sqp.tile([BL, O], F32, tag="d2")
            nc.vector.tensor_scalar_add(d2[:], sq[:], 1e-8)
            rt = sqp.tile([BL, O], F32, tag="rt")
            nc.scalar.sqrt(rt[:], d2[:])
            rs = sqp.tile([BL, O], F32, tag="rs")
            nc.vector.reciprocal(rs[:], rt[:])
            scale = sqp.tile([BL, O], F32, tag="scale")
            nc.vector.tensor_mul(scale[:], t[:], rs[:])
            nc.vector.tensor_mul(
                v_out.rearrange("p (d o) -> p d o", o=O),
                s_f32.rearrange("p (d o) -> p d o", o=O),
                _ap(scale[:], 0, [[0, D], [1, O]]),
            )

        def make_vexp(v_bf):
            """vexp[p=(n8,b), (d,o)] = v[b, (d,o)]; 8 sbuf->sbuf DMAs."""
            vx = vp.tile([128, OD], BF16, tag="vexp")
            for k in range(8):
                nc.sync.dma_start(
                    out=vx[k * BL : (k + 1) * BL, :], in_=v_bf[:]
                )
            return vx

        # ---------------- pass 1: s0 = sum_n u_hat / 32 ----------------
        s0_full = psum_s.tile([BL, 2, OD], F32, tag="sacc2")
        s0_ps = s0_full[:, 0, :]
        for ch in range(8):
            xt_t = xtp.tile([128, 16, BL], BF16)
            nc.sync.dma_start(
                out=xt_t[:], in_=xt8[:][:, ch * 16 : (ch + 1) * 16, :]
            )
            nc.sync.dma_start(
                out=w8_chunks[ch][:],
                in_=w8[:][:, ch * 16 : (ch + 1) * 16, :],
            )
            for j in range(16):
                j2 = ch * 16 + j
                nc.tensor.matmul(
                    s0_ps,
                    xt_t[:, j, :],
                    w8_sl(j2),
                    start=(j2 == 0),
                    stop=(j2 == J2 - 1),
                )
        s0_sb = sqp.tile([BL, OD], F32, tag="scopy")
        nc.scalar.activation(s0_sb[:], s0_ps, Copy)
        squash(s0_sb[:], v1_bf[:])
        vexp = make_vexp(v1_bf)

        # ---------------- passes 2, 3 (3-stage pipelined emission) ------
        NG = J2 // G
        for k in (1, 2):
            s_ps = psum_s.tile([BL, 2, OD], F32, name=f"sacc_{k}", tag="sacc2")
            state = {}

            def stage_a(g):
                """xbd loads + produce u + drain, for group g."""
                st = {"u": [], "xb": []}
                xb = xbp.tile([128, G, 128], BF16, tag="xb", name="xb")
                nc.sync.dma_start(
                    out=xb[:], in_=xbd8[:][:, g * G : (g + 1) * G, :]
                )
                for jj in range(G):
                    j2 = g * G + jj
                    u_ps = psum_u.tile([128, 2, D, O], F32)
                    for nb in range(2):
                        sl = slice(nb * 64, (nb + 1) * 64)
                        nc.tensor.matmul(
                            u_ps[:, nb, :, :],
                            xb[sl, jj, :],
                            w8_sl(j2, sl),
                            start=True,
                            stop=True,
                        )
                    u_sb = usbp.tile([128, 2, D, O], BF16, tag="usb")
                    nc.scalar.activation(u_sb[:], u_ps[:], Copy)
                    st["u"].append(u_sb)
                state[("a", g)] = st

            def stage_b(g):
                """q-mul, d-reduce (PE chains, both halves), exp, for group g."""
                st = state[("a", g)]
                ex = smp.tile([128, G, 2, O], BF16, tag="ex")
                q = qp.tile([128, G, 2, D, O], BF16, tag="q")
                for jj in range(G):
                    nc.vector.tensor_mul(
                        q[:, jj],
                        st["u"][jj][:],
                        _ap(vexp[:], 0, [[0, 2], [O, D], [1, O]]),
                    )
                # d-reduce: one PE identity-matmul chain per j2 covering both
                # nb halves (f=64 moving slices), exact f32 accumulation
                for jj in range(G):
                    a_ps = psum_a.tile([128, 512], F32, tag="aps")
                    for dd in range(D):
                        nc.tensor.matmul(
                            a_ps[:, 0 : 2 * O],
                            id_sb[:],
                            q[:, jj, :, dd, :],
                            start=(dd == 0),
                            stop=(dd == D - 1),
                        )
                    nc.scalar.activation(
                        ex[:, jj, :, :],
                        _ap(a_ps[:], 0, [[O, 2], [1, O]]),
                        Exp,
                    )
                st["ex"] = ex

            def stage_c(g):
                """softmax normalize + e-mul + n-reduce, for group g."""
                st = state.pop(("a", g))
                ex = st["ex"]
                se = smp.tile([128, G, 2], F32, tag="se")
                nc.vector.reduce_sum(out=se[:], in_=ex[:], axis=X)
                rse = smp.tile([128, G, 2], F32, tag="rse")
                nc.vector.reciprocal(rse[:], se[:])
                c_t = smp.tile([128, G, 2, O], BF16, tag="ct")
                nc.gpsimd.tensor_mul(
                    c_t[:], ex[:], _ap(rse[:], 0, [[2, G], [1, 2], [0, O]])
                )
                for jj in range(G):
                    j2 = g * G + jj
                    e_t = ep.tile([128, 2, D, O], BF16, tag="et")
                    nc.vector.tensor_mul(
                        e_t[:],
                        st["u"][jj][:],
                        _ap(c_t[:], jj * 2 * O, [[O, 2], [0, D], [1, O]]),
                    )
                    for nb in range(2):
                        nc.tensor.matmul(
                            s_ps[:, nb, :],
                            ones_sb[:],
                            e_t[:, nb, :, :],
                            start=(j2 == 0),
                            stop=(j2 == J2 - 1),
                        )

            for g in range(NG + 2):
                if g >= 2:
                    stage_c(g - 2)
                if 1 <= g <= NG:
                    stage_b(g - 1)
                if g < NG:
                    stage_a(g)

            # s = nb0 + nb1, squash (only one PSUM operand allowed per op)
            s1_sb = sqp.tile([BL, OD], F32, tag="s1copy")
            nc.scalar.activation(s1_sb[:], s_ps[:, 1, :], Copy)
            s_sb = sqp.tile([BL, OD], F32, tag="scopy")
            nc.vector.tensor_add(s_sb[:], s_ps[:, 0, :], s1_sb[:])
            if k == 1:
                v2_bf = vp.tile([BL, OD], BF16, tag="v2")
                squash(s_sb[:], v2_bf[:])
                nc.vector.tensor_add(v12_bf[:], v1_bf[:], v2_bf[:])
                vexp = make_vexp(v12_bf)
            else:
                v_f32 = sqp.tile([BL, OD], F32, tag="vf32")
                squash(s_sb[:], v_f32[:])
                nc.sync.dma_start(out=out[:], in_=v_f32[:])

    nc.compile()
    return nc


_nc_cache = {}


def _get_nc():
    if "nc" not in _nc_cache:
        _nc_cache["nc"] = build_nc()
    return _nc_cache["nc"]


def _prep_host(x, W):
    """Build the per-core input maps (numpy only)."""
    arrW = W.reshape(J2, 2, 8, O, I, D)  # [j2, nb, n8, o, i, d]
    W8h = arrW.transpose(1, 2, 4, 0, 5, 3).reshape(128, J2, OD).astype(_BF)
    ones8h = np.zeros((128, BL), dtype=_BF)
    for p in range(128):
        ones8h[p, p % BL] = 1.0
    identh = np.eye(128, dtype=_BF)
    in_maps = []
    for c in range(CORES):
        xl = x[c * BL : (c + 1) * BL]  # [16, 2048, 8]
        arr = xl.reshape(BL, J2, 2, 8, I)  # [b, j2, nb, n8, i]
        t = arr.transpose(2, 3, 4, 1, 0)  # [nb, n8, i, j2, b]
        xt8h = (t / 32.0).reshape(128, J2, BL).astype(_BF)
        xbd = np.zeros((2, 8, I, J2, 8, BL), dtype=np.float32)
        for n8 in range(8):
            xbd[:, n8, :, :, n8, :] = t[:, n8]
        xbd8h = xbd.reshape(128, J2, 128).astype(_BF)
        in_maps.append(
            {
                "w8": W8h,
                "xt8": xt8h,
                "xbd8": xbd8h,
                "ones8": ones8h,
                "ident": identh,
            }
        )
    return in_maps


TRACE = False
_last = {}


def kernel(x: np.ndarray, W: np.ndarray) -> np.ndarray:
    nc = _get_nc()
    in_maps = _prep_host(
        np.asarray(x, dtype=np.float32), np.asarray(W, dtype=np.float32)
    )
    res = run_bass_kernel_spmd(
        nc, in_maps, core_ids=list(range(CORES)), trace=TRACE
    )
    _last["res"] = res
    outs = [
        r["out"].reshape(BL, D, O).transpose(0, 2, 1) for r in res.results
    ]
    return np.concatenate(outs, axis=0).astype(np.float32)


if __name__ == "__main__":
    rng = np.random.default_rng(0)
    x = rng.standard_normal((B, N, I), dtype=np.float32)
    W = rng.standard_normal((N, O, I, D), dtype=np.float32)
    v = kernel(x, W)
    print(v.shape, v.dtype, float(np.abs(v).mean()))


# revision 5
# speedup vs baseline: 1.2169x; 1.0038x over previous
"""CapsuleLayer (dynamic routing) Trainium2 kernel, v14 (pipelined emission).

x: [128, 2048, 8] f32, W: [2048, 32, 8, 16] f32 -> v: [128, 32, 16] f32

Batch split across 8 cores (16 each), W replicated in SBUF (bf16).
Layout: partition p = (n8, b16); u free = (nb2, d16, o32). Logits are
linear in v so pass 3 uses vexp(v1+v2); no bias tensor is stored.

Per j2 (16 input caps): PE produces u (two k=64 matmuls) and reduces n
(ones matmul); ACT drains PSUM and does exp; DVE does the two big
elementwise muls at 2x plus softmax smalls; the d-reduce splits between a
PE identity-matmul chain (nb=0, PSUM) and a gpsimd add-tree (nb=1, SBUF).

Instruction emission is software-pipelined in three stages (produce |
q+dred | softmax+e+ones) staggered by one j2-group each so no engine
sequencer blocks on a just-issued cross-engine dependency.
"""

from contextlib import ExitStack

import numpy as np
import ml_dtypes

import concourse.bass as bass
import concourse.bacc as bacc
import concourse.tile as tile
from concourse import mybir
from concourse.bass_utils import run_bass_kernel_spmd

BF16 = mybir.dt.bfloat16
F32 = mybir.dt.float32
X = mybir.AxisListType.X
Exp = mybir.ActivationFunctionType.Exp
Copy = mybir.ActivationFunctionType.Copy
MULT = mybir.AluOpType.mult
ADD = mybir.AluOpType.add

B, N, O, I, D = 128, 2048, 32, 8, 16
CORES = 8
BL = B // CORES            # 16 batch elements per core
J2 = N // 16               # 128 groups of 16 input caps
OD = O * D                 # 512
G = 4                      # j2 group size for batched softmax

_BF = ml_dtypes.bfloat16


def _ap(t_ap, extra_offset, dims):
    """AP with the same tensor/partition dim but custom free dims."""
    return bass.AP(
        tensor=t_ap.tensor,
        offset=t_ap.offset + extra_offset,
        ap=[list(t_ap.ap[0])] + [list(d) for d in dims],
    )


def build_nc():
    nc = bacc.Bacc("TRN2", target_bir_lowering=False)

    w8 = nc.dram_tensor("w8", [128, J2, OD], BF16, kind="ExternalInput")
    xt8 = nc.dram_tensor("xt8", [128, J2, BL], BF16, kind="ExternalInput")
    xbd8 = nc.dram_tensor("xbd8", [128, J2, 128], BF16, kind="ExternalInput")
    ones8 = nc.dram_tensor("ones8", [128, BL], BF16, kind="ExternalInput")
    ident = nc.dram_tensor("ident", [128, 128], BF16, kind="ExternalInput")
    out = nc.dram_tensor("out", [BL, OD], F32, kind="ExternalOutput")

    with tile.TileContext(nc) as tc, ExitStack() as ctx:
        const = ctx.enter_context(tc.tile_pool(name="const", bufs=1))
        xtp = ctx.enter_context(tc.tile_pool(name="xtp", bufs=2))
        xbp = ctx.enter_context(tc.tile_pool(name="xbp", bufs=4))
        vp = ctx.enter_context(tc.tile_pool(name="vp", bufs=1))
        usbp = ctx.enter_context(tc.tile_pool(name="usbp", bufs=11))
        qp = ctx.enter_context(tc.tile_pool(name="qp", bufs=2))
        tp = ctx.enter_context(tc.tile_pool(name="tp", bufs=3))
        ep = ctx.enter_context(tc.tile_pool(name="ep", bufs=3))
        smp = ctx.enter_context(tc.tile_pool(name="smp", bufs=3))
        sqp = ctx.enter_context(tc.tile_pool(name="sqp", bufs=1))
        psum_u = ctx.enter_context(tc.tile_pool(name="psum_u", bufs=2, space="PSUM"))
        psum_a = ctx.enter_context(tc.tile_pool(name="psum_a", bufs=2, space="PSUM"))
        psum_s = ctx.enter_context(tc.tile_pool(name="psum_s", bufs=1, space="PSUM"))

        w8_chunks = []
        for ch in range(8):
            w8_c = const.tile([128, 16, OD], BF16, name=f"w8_{ch}", tag=f"w8_{ch}")
            w8_chunks.append(w8_c)

        def w8_sl(j2, sl=slice(None)):
            return w8_chunks[j2 // 16][sl, j2 % 16, :]
        ones_sb = const.tile([128, BL], BF16)
        nc.sync.dma_start(out=ones_sb[:], in_=ones8[:])
        id_sb = const.tile([128, 128], BF16)
        nc.sync.dma_start(out=id_sb[:], in_=ident[:])

        # v holders ([16, (d, o)])
        v1_bf = vp.tile([BL, OD], BF16, tag="v1")
        v12_bf = vp.tile([BL, OD], BF16, tag="v12")

        def squash(s_f32, v_out):
            """v_out(bf16) = squash(s) over d; s_f32: [16, (d,o)] f32 sbuf."""
            ssq = sqp.tile([BL, D, O], F32, tag="ssq")
            nc.vector.tensor_mul(
                ssq[:],
                s_f32.rearrange("p (d o) -> p d o", o=O),
                s_f32.rearrange("p (d o) -> p d o", o=O),
            )
            sq = sqp.tile([BL, O], F32, tag="sq")
            nc.vector.reduce_sum(
                out=sq[:], in_=ssq[:].rearrange("p d o -> p o d"), axis=X
            )
            d1 = sqp.tile([BL, O], F32, tag="d1")
            nc.vector.tensor_scalar_add(d1[:], sq[:], 1.0)
            r1 = sqp.tile([BL, O], F32, tag="r1")
            nc.vector.reciprocal(r1[:], d1[:])
            t = sqp.tile([BL, O], F32, tag="t")
            nc.vector.tensor_mul(t[:], sq[:], r1[:])
            d2 = sqp.tile([BL, O], F32, tag="d2")
            nc.vector.tensor_scalar_add(d2[:], sq[:], 1e-8)
            rt = sqp.tile([BL, O], F32, tag="rt")
            nc.scalar.sqrt(rt[:], d2[:])
            rs = sqp.tile([BL, O], F32, tag="rs")
            nc.vector.reciprocal(rs[:], rt[:])
            scale = sqp.tile([BL, O], F32, tag="scale")
            nc.vector.tensor_mul(scale[:], t[:], rs[:])
            nc.vector.tensor_mul(
                v_out.rearrange("p (d o) -> p d o", o=O),
                s_f32.rearrange("p (d o) -> p d o", o=O),
                _ap(scale[:], 0, [[0, D], [1, O]]),
            )

        def make_vexp(v_bf):
            """vexp[p=(n8,b), (d,o)] = v[b, (d,o)]; 8 sbuf->sbuf DMAs."""
            vx = vp.tile([128, OD], BF16, tag="vexp")
            for k in range(8):
                nc.sync.dma_start(
                    out=vx[k * BL : (k + 1) * BL, :], in_=v_bf[:]
                )
            return vx

        # ---------------- pass 1: s0 = sum_n u_hat / 32 ----------------
        s0_full = psum_s.tile([BL, 2, OD], F32, tag="sacc2")
        s0_ps = s0_full[:, 0, :]
        for ch in range(8):
            xt_t = xtp.tile([128, 16, BL], BF16)
            nc.sync.dma_start(
                out=xt_t[:], in_=xt8[:][:, ch * 16 : (ch + 1) * 16, :]
            )
            nc.sync.dma_start(
                out=w8_chunks[ch][:],
                in_=w8[:][:, ch * 16 : (ch + 1) * 16, :],
            )
            for j in range(16):
                j2 = ch * 16 + j
                nc.tensor.matmul(
                    s0_ps,
                    xt_t[:, j, :],
                    w8_sl(j2),
                    start=(j2 == 0),
                    stop=(j2 == J2 - 1),
                )
        s0_sb = sqp.tile([BL, OD], F32, tag="scopy")
        nc.scalar.activation(s0_sb[:], s0_ps, Copy)
        squash(s0_sb[:], v1_bf[:])
        vexp = make_vexp(v1_bf)

        # ---------------- passes 2, 3 (3-stage pipelined emission) ------
        NG = J2 // G
        for k in (1, 2):
            s_ps = psum_s.tile([BL, 2, OD], F32, name=f"sacc_{k}", tag="sacc2")
            state = {}

            def stage_a(g):
                """xbd loads + produce u + drain, for group g."""
                st = {"u": [], "xb": []}
                xb = xbp.tile([128, G, 128], BF16, tag="xb", name="xb")
                nc.sync.dma_start(
                    out=xb[:], in_=xbd8[:][:, g * G : (g + 1) * G, :]
                )
                for jj in range(G):
                    j2 = g * G + jj
                    u_ps = psum_u.tile([128, 2, D, O], F32)
                    for nb in range(2):
                        sl = slice(nb * 64, (nb + 1) * 64)
                        nc.tensor.matmul(
                            u_ps[:, nb, :, :],
                            xb[sl, jj, :],
                            w8_sl(j2, sl),
                            start=True,
                            stop=True,
                        )
                    u_sb = usbp.tile([128, 2, D, O], BF16, tag="usb")
                    nc.scalar.activation(u_sb[:], u_ps[:], Copy)
                    st["u"].append(u_sb)
                state[("a", g)] = st

            def stage_b_q(g):
                """q-mul for group g (runs while prior group's softmax sits
                on Pool/ACT)."""
                st = state[("a", g)]
                q = qp.tile([128, G, 2, D, O], BF16, tag="q")
                for jj in range(G):
                    nc.vector.tensor_mul(
                        q[:, jj],
                        st["u"][jj][:],
                        _ap(vexp[:], 0, [[0, 2], [O, D], [1, O]]),
                    )
                st["q"] = q

            def stage_b_red(g):
                """d-reduce + exp for group g."""
                st = state[("a", g)]
                ex = smp.tile([128, G, 2, O], BF16, tag="ex")
                q = st["q"]
                # d-reduce: one PE identity-matmul chain for the whole group
                # (f=64 moving slices, disjoint 64-col regions of one PSUM
                # bank), exact f32 accumulation, one batched exp
                a_ps = psum_a.tile([128, 512], F32, tag="aps")
                for jj in range(G):
                    for dd in range(D):
                        nc.tensor.matmul(
                            a_ps[:, jj * 2 * O : (jj + 1) * 2 * O],
                            id_sb[:],
                            q[:, jj, :, dd, :],
                            start=(jj == 0 and dd == 0),
                            stop=(jj == G - 1 and dd == D - 1),
                        )
                nc.scalar.activation(
                    ex[:],
                    _ap(a_ps[:], 0, [[2 * O, G], [O, 2], [1, O]]),
                    Exp,
                )
                st["ex"] = ex

            def stage_c_sm(g):
                """softmax normalize for group g."""
                st = state[("a", g)]
                ex = st["ex"]
                se = smp.tile([128, G, 2], F32, tag="se")
                nc.vector.reduce_sum(out=se[:], in_=ex[:], axis=X)
                rse = smp.tile([128, G, 2], F32, tag="rse")
                nc.vector.reciprocal(rse[:], se[:])
                c_t = smp.tile([128, G, 2, O], BF16, tag="ct")
                nc.gpsimd.tensor_mul(
                    c_t[:], ex[:], _ap(rse[:], 0, [[2, G], [1, 2], [0, O]])
                )
                st["ct"] = c_t

            def stage_c_eo(g):
                """e-mul + n-reduce for group g."""
                st = state.pop(("a", g))
                c_t = st["ct"]
                for jj in range(G):
                    j2 = g * G + jj
                    e_t = ep.tile([128, 2, D, O], BF16, tag="et")
                    nc.vector.tensor_mul(
                        e_t[:],
                        st["u"][jj][:],
                        _ap(c_t[:], jj * 2 * O, [[O, 2], [0, D], [1, O]]),
                    )
                    for nb in range(2):
                        nc.tensor.matmul(
                            s_ps[:, nb, :],
                            ones_sb[:],
                            e_t[:, nb, :, :],
                            start=(j2 == 0),
                            stop=(j2 == J2 - 1),
                        )

            for g in range(NG + 2):
                if g >= 2:
                    stage_c_sm(g - 2)
                if 1 <= g <= NG:
                    stage_b_q(g - 1)
                    stage_b_red(g - 1)
                if g >= 2:
                    stage_c_eo(g - 2)
                if g < NG:
                    stage_a(g)

            # s = nb0 + nb1, squash (only one PSUM operand allowed per op)
            s1_sb = sqp.tile([BL, OD], F32, tag="s1copy")
            nc.scalar.activation(s1_sb[:], s_ps[:, 1, :], Copy)
            s_sb = sqp.tile([BL, OD], F32, tag="scopy")
            nc.vector.tensor_add(s_sb[:], s_ps[:, 0, :], s1_sb[:])
            if k == 1:
                v2_bf = vp.tile([BL, OD], BF16, tag="v2")
                squash(s_sb[:], v2_bf[:])
                nc.vector.tensor_add(v12_bf[:], v1_bf[:], v2_bf[:])
                vexp = make_vexp(v12_bf)
            else:
                v_f32 = sqp.tile([BL, OD], F32, tag="vf32")
                squash(s_sb[:], v_f32[:])
                nc.sync.dma_start(out=out[:], in_=v_f32[:])

    nc.compile()
    return nc


_nc_cache = {}


def _get_nc():
    if "nc" not in _nc_cache:
        _nc_cache["nc"] = build_nc()
    return _nc_cache["nc"]


def _prep_host(x, W):
    """Build the per-core input maps (numpy only)."""
    arrW = W.reshape(J2, 2, 8, O, I, D)  # [j2, nb, n8, o, i, d]
    W8h = arrW.transpose(1, 2, 4, 0, 5, 3).reshape(128, J2, OD).astype(_BF)
    ones8h = np.zeros((128, BL), dtype=_BF)
    for p in range(128):
        ones8h[p, p % BL] = 1.0
    identh = np.eye(128, dtype=_BF)
    in_maps = []
    for c in range(CORES):
        xl = x[c * BL : (c + 1) * BL]  # [16, 2048, 8]
        arr = xl.reshape(BL, J2, 2, 8, I)  # [b, j2, nb, n8, i]
        t = arr.transpose(2, 3, 4, 1, 0)  # [nb, n8, i, j2, b]
        xt8h = (t / 32.0).reshape(128, J2, BL).astype(_BF)
        xbd = np.zeros((2, 8, I, J2, 8, BL), dtype=np.float32)
        for n8 in range(8):
            xbd[:, n8, :, :, n8, :] = t[:, n8]
        xbd8h = xbd.reshape(128, J2, 128).astype(_BF)
        in_maps.append(
            {
                "w8": W8h,
                "xt8": xt8h,
                "xbd8": xbd8h,
                "ones8": ones8h,
                "ident": identh,
            }
        )
    return in_maps


TRACE = False
_last = {}


def kernel(x: np.ndarray, W: np.ndarray) -> np.ndarray:
    nc = _get_nc()
    in_maps = _prep_host(
        np.asarray(x, dtype=np.float32), np.asarray(W, dtype=np.float32)
    )
    res = run_bass_kernel_spmd(
        nc, in_maps, core_ids=list(range(CORES)), trace=TRACE
    )
    _last["res"] = res
    outs = [
        r["out"].reshape(BL, D, O).transpose(0, 2, 1) for r in res.results
    ]
    return np.concatenate(outs, axis=0).astype(np.float32)


if __name__ == "__main__":
    rng = np.random.default_rng(0)
    x = rng.standard_normal((B, N, I), dtype=np.float32)
    W = rng.standard_normal((N, O, I, D), dtype=np.float32)
    v = kernel(x, W)
    print(v.shape, v.dtype, float(np.abs(v).mean()))


# revision 6
# speedup vs baseline: 1.2300x; 1.0108x over previous
"""CapsuleLayer (dynamic routing) Trainium2 kernel, v17 (pipelined emission).

x: [128, 2048, 8] f32, W: [2048, 32, 8, 16] f32 -> v: [128, 32, 16] f32

Batch split across 8 cores (16 each), W replicated in SBUF (bf16).
Layout: partition p = (n8, b16); u free = (nb2, d16, o32). Logits are
linear in v so pass 3 uses vexp(v1+v2); no bias tensor is stored.

Per j2 (16 input caps): PE produces u (two k=64 matmuls) and reduces n
(ones matmul); ACT drains PSUM and does exp; DVE does the two big
elementwise muls at 2x plus softmax smalls; the d-reduce splits between a
PE identity-matmul chain (nb=0, PSUM) and a gpsimd add-tree (nb=1, SBUF).

Instruction emission is software-pipelined in three stages (produce |
q+dred | softmax+e+ones) staggered by one j2-group each so no engine
sequencer blocks on a just-issued cross-engine dependency.
"""

from contextlib import ExitStack

import numpy as np
import ml_dtypes

import concourse.bass as bass
import concourse.bacc as bacc
import concourse.tile as tile
from concourse import mybir
from concourse.bass_utils import run_bass_kernel_spmd

BF16 = mybir.dt.bfloat16
F32 = mybir.dt.float32
X = mybir.AxisListType.X
Exp = mybir.ActivationFunctionType.Exp
Copy = mybir.ActivationFunctionType.Copy
MULT = mybir.AluOpType.mult
ADD = mybir.AluOpType.add

B, N, O, I, D = 128, 2048, 32, 8, 16
CORES = 8
BL = B // CORES            # 16 batch elements per core
J2 = N // 16               # 128 groups of 16 input caps
OD = O * D                 # 512
G = 4                      # j2 group size for batched softmax

_BF = ml_dtypes.bfloat16


def _ap(t_ap, extra_offset, dims):
    """AP with the same tensor/partition dim but custom free dims."""
    return bass.AP(
        tensor=t_ap.tensor,
        offset=t_ap.offset + extra_offset,
        ap=[list(t_ap.ap[0])] + [list(d) for d in dims],
    )


def build_nc():
    nc = bacc.Bacc("TRN2", target_bir_lowering=False)

    w8 = nc.dram_tensor("w8", [128, J2, OD], BF16, kind="ExternalInput")
    xt8 = nc.dram_tensor("xt8", [128, J2, BL], BF16, kind="ExternalInput")
    xbd8 = nc.dram_tensor("xbd8", [128, J2, 128], BF16, kind="ExternalInput")
    ones8 = nc.dram_tensor("ones8", [128, BL], BF16, kind="ExternalInput")
    ident = nc.dram_tensor("ident", [128, 128], BF16, kind="ExternalInput")
    out = nc.dram_tensor("out", [BL, OD], F32, kind="ExternalOutput")

    with tile.TileContext(nc) as tc, ExitStack() as ctx:
        const = ctx.enter_context(tc.tile_pool(name="const", bufs=1))
        xtp = ctx.enter_context(tc.tile_pool(name="xtp", bufs=2))
        xbp = ctx.enter_context(tc.tile_pool(name="xbp", bufs=4))
        vp = ctx.enter_context(tc.tile_pool(name="vp", bufs=1))
        usbp = ctx.enter_context(tc.tile_pool(name="usbp", bufs=11))
        qp = ctx.enter_context(tc.tile_pool(name="qp", bufs=2))
        tp = ctx.enter_context(tc.tile_pool(name="tp", bufs=3))
        ep = ctx.enter_context(tc.tile_pool(name="ep", bufs=3))
        smp = ctx.enter_context(tc.tile_pool(name="smp", bufs=3))
        sqp = ctx.enter_context(tc.tile_pool(name="sqp", bufs=1))
        psum_u = ctx.enter_context(tc.tile_pool(name="psum_u", bufs=2, space="PSUM"))
        psum_a = ctx.enter_context(tc.tile_pool(name="psum_a", bufs=3, space="PSUM"))
        psum_s = ctx.enter_context(tc.tile_pool(name="psum_s", bufs=1, space="PSUM"))

        w8_chunks = []
        for ch in range(8):
            w8_c = const.tile([128, 16, OD], BF16, name=f"w8_{ch}", tag=f"w8_{ch}")
            w8_chunks.append(w8_c)

        def w8_sl(j2, sl=slice(None)):
            return w8_chunks[j2 // 16][sl, j2 % 16, :]
        ones_sb = const.tile([128, BL], BF16)
        nc.sync.dma_start(out=ones_sb[:], in_=ones8[:])
        id_sb = const.tile([128, 128], BF16)
        nc.sync.dma_start(out=id_sb[:], in_=ident[:])

        # v holders ([16, (d, o)])
        v1_bf = vp.tile([BL, OD], BF16, tag="v1")
        v12_bf = vp.tile([BL, OD], BF16, tag="v12")

        def squash(s_f32, v_out):
            """v_out(bf16) = squash(s) over d; s_f32: [16, (d,o)] f32 sbuf."""
            ssq = sqp.tile([BL, D, O], F32, tag="ssq")
            nc.vector.tensor_mul(
                ssq[:],
                s_f32.rearrange("p (d o) -> p d o", o=O),
                s_f32.rearrange("p (d o) -> p d o", o=O),
            )
            sq = sqp.tile([BL, O], F32, tag="sq")
            nc.vector.reduce_sum(
                out=sq[:], in_=ssq[:].rearrange("p d o -> p o d"), axis=X
            )
            d1 = sqp.tile([BL, O], F32, tag="d1")
            nc.vector.tensor_scalar_add(d1[:], sq[:], 1.0)
            r1 = sqp.tile([BL, O], F32, tag="r1")
            nc.vector.reciprocal(r1[:], d1[:])
            t = sqp.tile([BL, O], F32, tag="t")
            nc.vector.tensor_mul(t[:], sq[:], r1[:])
            d2 = sqp.tile([BL, O], F32, tag="d2")
            nc.vector.tensor_scalar_add(d2[:], sq[:], 1e-8)
            rt = sqp.tile([BL, O], F32, tag="rt")
            nc.scalar.sqrt(rt[:], d2[:])
            rs = sqp.tile([BL, O], F32, tag="rs")
            nc.vector.reciprocal(rs[:], rt[:])
            scale = sqp.tile([BL, O], F32, tag="scale")
            nc.vector.tensor_mul(scale[:], t[:], rs[:])
            nc.vector.tensor_mul(
                v_out.rearrange("p (d o) -> p d o", o=O),
                s_f32.rearrange("p (d o) -> p d o", o=O),
                _ap(scale[:], 0, [[0, D], [1, O]]),
            )

        def make_vexp(v_bf):
            """vexp[p=(n8,b), (d,o)] = v[b, (d,o)]; 8 sbuf->sbuf DMAs."""
            vx = vp.tile([128, OD], BF16, tag="vexp")
            for k in range(8):
                nc.sync.dma_start(
                    out=vx[k * BL : (k + 1) * BL, :], in_=v_bf[:]
                )
            return vx

        # ---------------- pass 1: s0 = sum_n u_hat / 32 ----------------
        s0_full = psum_s.tile([BL, OD], F32, tag="sacc2")
        s0_ps = s0_full[:]
        for ch in range(8):
            xt_t = xtp.tile([128, 16, BL], BF16)
            nc.sync.dma_start(
                out=xt_t[:], in_=xt8[:][:, ch * 16 : (ch + 1) * 16, :]
            )
            nc.sync.dma_start(
                out=w8_chunks[ch][:],
                in_=w8[:][:, ch * 16 : (ch + 1) * 16, :],
            )
            for j in range(16):
                j2 = ch * 16 + j
                nc.tensor.matmul(
                    s0_ps,
                    xt_t[:, j, :],
                    w8_sl(j2),
                    start=(j2 == 0),
                    stop=(j2 == J2 - 1),
                )
        s0_sb = sqp.tile([BL, OD], F32, tag="scopy")
        nc.scalar.activation(s0_sb[:], s0_ps, Copy)
        squash(s0_sb[:], v1_bf[:])
        vexp = make_vexp(v1_bf)

        # ---------------- passes 2, 3 (3-stage pipelined emission) ------
        NG = J2 // G
        for k in (1, 2):
            s_ps = psum_s.tile([BL, OD], F32, name=f"sacc_{k}", tag="sacc2")
            state = {}

            def stage_a(g):
                """xbd loads + produce u + drain, for group g."""
                st = {"u": [], "xb": []}
                xb = xbp.tile([128, G, 128], BF16, tag="xb", name="xb")
                nc.sync.dma_start(
                    out=xb[:], in_=xbd8[:][:, g * G : (g + 1) * G, :]
                )
                for jj in range(G):
                    j2 = g * G + jj
                    u_ps = psum_u.tile([128, 2, D, O], F32)
                    for nb in range(2):
                        sl = slice(nb * 64, (nb + 1) * 64)
                        nc.tensor.matmul(
                            u_ps[:, nb, :, :],
                            xb[sl, jj, :],
                            w8_sl(j2, sl),
                            start=True,
                            stop=True,
                        )
                    u_sb = usbp.tile([128, 2, D, O], BF16, tag="usb")
                    nc.scalar.activation(u_sb[:], u_ps[:], Copy)
                    st["u"].append(u_sb)
                state[("a", g)] = st

            def stage_b_q(g):
                """q-mul for group g (runs while prior group's softmax sits
                on Pool/ACT)."""
                st = state[("a", g)]
                q = qp.tile([128, G, 2, D, O], BF16, tag="q")
                for jj in range(G):
                    nc.vector.tensor_mul(
                        q[:, jj],
                        st["u"][jj][:],
                        _ap(vexp[:], 0, [[0, 2], [O, D], [1, O]]),
                    )
                st["q"] = q

            def stage_b_red(g):
                """d-reduce + exp for group g."""
                st = state[("a", g)]
                ex = smp.tile([128, G, 2, O], BF16, tag="ex")
                q = st["q"]
                # d-reduce: one PE identity-matmul chain for the whole group
                # (f=64 moving slices, disjoint 64-col regions of one PSUM
                # bank), exact f32 accumulation, one batched exp
                a_ps = psum_a.tile([128, 512], F32, tag="aps")
                for jj in range(G):
                    for dd in range(D):
                        nc.tensor.matmul(
                            a_ps[:, jj * 2 * O : (jj + 1) * 2 * O],
                            id_sb[:],
                            q[:, jj, :, dd, :],
                            start=(jj == 0 and dd == 0),
                            stop=(jj == G - 1 and dd == D - 1),
                        )
                nc.scalar.activation(
                    ex[:],
                    _ap(a_ps[:], 0, [[2 * O, G], [O, 2], [1, O]]),
                    Exp,
                )
                st["ex"] = ex

            def stage_c_sm(g):
                """softmax normalize for group g."""
                st = state[("a", g)]
                ex = st["ex"]
                se = smp.tile([128, G, 2], F32, tag="se")
                nc.vector.reduce_sum(out=se[:], in_=ex[:], axis=X)
                rse = smp.tile([128, G, 2], F32, tag="rse")
                nc.vector.reciprocal(rse[:], se[:])
                c_t = smp.tile([128, G, 2, O], BF16, tag="ct")
                nc.gpsimd.tensor_mul(
                    c_t[:], ex[:], _ap(rse[:], 0, [[2, G], [1, 2], [0, O]])
                )
                st["ct"] = c_t

            def stage_c_eo(g):
                """e-mul + n-reduce for group g."""
                st = state.pop(("a", g))
                c_t = st["ct"]
                for jj in range(G):
                    j2 = g * G + jj
                    e_t = ep.tile([128, 2, D, O], BF16, tag="et")
                    nc.vector.tensor_mul(
                        e_t[:],
                        st["u"][jj][:],
                        _ap(c_t[:], jj * 2 * O, [[O, 2], [0, D], [1, O]]),
                    )
                    for nb in range(2):
                        nc.tensor.matmul(
                            s_ps[:],
                            ones_sb[:],
                            e_t[:, nb, :, :],
                            start=(j2 == 0 and nb == 0),
                            stop=(j2 == J2 - 1 and nb == 1),
                        )

            for g in range(NG + 2):
                if g >= 2:
                    stage_c_sm(g - 2)
                if 1 <= g <= NG:
                    stage_b_q(g - 1)
                    stage_b_red(g - 1)
                if g >= 2:
                    stage_c_eo(g - 2)
                if g < NG:
                    stage_a(g)

            s_sb = sqp.tile([BL, OD], F32, tag="scopy")
            nc.scalar.activation(s_sb[:], s_ps[:], Copy)
            if k == 1:
                v2_bf = vp.tile([BL, OD], BF16, tag="v2")
                squash(s_sb[:], v2_bf[:])
                nc.vector.tensor_add(v12_bf[:], v1_bf[:], v2_bf[:])
                vexp = make_vexp(v12_bf)
            else:
                v_f32 = sqp.tile([BL, OD], F32, tag="vf32")
                squash(s_sb[:], v_f32[:])
                nc.sync.dma_start(out=out[:], in_=v_f32[:])

    nc.compile()
    return nc


_nc_cache = {}


def _get_nc():
    if "nc" not in _nc_cache:
        _nc_cache["nc"] = build_nc()
    return _nc_cache["nc"]


def _prep_host(x, W):
    """Build the per-core input maps (numpy only)."""
    arrW = W.reshape(J2, 2, 8, O, I, D)  # [j2, nb, n8, o, i, d]
    W8h = arrW.transpose(1, 2, 4, 0, 5, 3).reshape(128, J2, OD).astype(_BF)
    ones8h = np.zeros((128, BL), dtype=_BF)
    for p in range(128):
        ones8h[p, p % BL] = 1.0
    identh = np.eye(128, dtype=_BF)
    in_maps = []
    for c in range(CORES):
        xl = x[c * BL : (c + 1) * BL]  # [16, 2048, 8]
        arr = xl.reshape(BL, J2, 2, 8, I)  # [b, j2, nb, n8, i]
        t = arr.transpose(2, 3, 4, 1, 0)  # [nb, n8, i, j2, b]
        xt8h = (t / 32.0).reshape(128, J2, BL).astype(_BF)
        xbd = np.zeros((2, 8, I, J2, 8, BL), dtype=np.float32)
        for n8 in range(8):
            xbd[:, n8, :, :, n8, :] = t[:, n8]
        xbd8h = xbd.reshape(128, J2, 128).astype(_BF)
        in_maps.append(
            {
                "w8": W8h,
                "xt8": xt8h,
                "xbd8": xbd8h,
                "ones8": ones8h,
                "ident": identh,
            }
        )
    return in_maps


TRACE = False
_last = {}


def kernel(x: np.ndarray, W: np.ndarray) -> np.ndarray:
    nc = _get_nc()
    in_maps = _prep_host(
        np.asarray(x, dtype=np.float32), np.asarray(W, dtype=np.float32)
    )
    res = run_bass_kernel_spmd(
        nc, in_maps, core_ids=list(range(CORES)), trace=TRACE
    )
    _last["res"] = res
    outs = [
        r["out"].reshape(BL, D, O).transpose(0, 2, 1) for r in res.results
    ]
    return np.concatenate(outs, axis=0).astype(np.float32)


if __name__ == "__main__":
    rng = np.random.default_rng(0)
    x = rng.standard_normal((B, N, I), dtype=np.float32)
    W = rng.standard_normal((N, O, I, D), dtype=np.float32)
    v = kernel(x, W)
    print(v.shape, v.dtype, float(np.abs(v).mean()))
